# revision 36
# baseline (speedup 1.0000x reference)
"""Trainium2 Bass kernel for the gnn_message_passing ConvLayer problem.

Computes, for feature i in 0..4 and rotation r in 0..15:
    g      = exp(-(rho-mu_r)^2/cr) * exp(-(mod(theta+rot_r)-mu_t)^2/ct) * mask
    g      = g / (sum_k g + eps)
    desc   = einsum('svk,sv->sk', g, feats_i)
    conv   = desc @ W_i + b_i
    out_i  = relu(max_r conv)            # [NS, K]
out = stack(out_i, axis=2).reshape(NS, K*F)

Sharding: data-parallel over the NS=512 sample axis across 8 NeuronCores
(64 samples/core); the tiny per-feature params are replicated. No cross-core
communication.

Fast path: setup_inputs() builds mu/sigma as a tiled 5x16 (rho x theta) grid
shared across features, so the Gaussian factorises:
    g[..,k] = g_rho[.., k//16] * g_th[.., k%16]
The normaliser sum_k g = Sr*St also factorises, and eps (1e-5) is negligible
against Sr*St >= ~0.07, so 1/(Sr*St+eps) ~= (1/Sr)*(1/St).  That makes the
moving operand of the vertex-contraction matmul rotation-INDEPENDENT:
    u[v,(i,kr)|s]    = feats*mask*(1/Sr)*g_rho          (tiny, built once)
    gthn[v,kt|r,s]   = g_th*(1/St)                      (one big DVE pass)
    desc[kt,(i,kr)]  = gthn^T @ u     per (r,s) pair    (bf16 matmuls)
    conv             = accumulate over kr of W-slice^T @ desc (bf16, N=256)
Engine split: Act does the (T-mu)^2 (fused bias) + exp; Pool does the big
St reduction and PSUM->SBUF copies; DVE does the normalise multiply and the
small elementwise work; PE does desc/conv matmuls and transposes.
This structure is detected at runtime (exact equality checks on the host);
arbitrary parameters fall back to a generic kernel.
"""

import math
import sys

import numpy as np

sys.path.insert(0, "/opt/trn_rl_repo")

import ml_dtypes

import concourse.bacc as bacc
import concourse.bass as bass
import concourse.tile as tile
from concourse import mybir
from concourse.bass_utils import run_bass_kernel_spmd

F32 = mybir.dt.float32
BF16 = mybir.dt.bfloat16
AF = mybir.ActivationFunctionType
ALU = mybir.AluOpType
AX = mybir.AxisListType

N_CORES = 8
NS = 512
NV = 128          # vertices / patch == SBUF partition count
NF = 5            # features
K = 80            # gaussian kernels
R = 16            # rotations
NT = 16           # theta centers (fast path)
NR = 5            # rho centers (fast path)
SC = NS // N_CORES  # samples per core
EPS = 1e-5
TWO_PI = 2.0 * math.pi


def _bcast(ap, dim, shape):
    """Insert a stride-0 dim at `dim` and broadcast to `shape`."""
    return ap.unsqueeze(dim).to_broadcast(shape)


def _dma_bcast_rows(nc, out_sb, dram_handle, n_free):
    """DMA a [1, n_free] DRAM tensor into all partitions of out_sb."""
    src = dram_handle[:]
    ap = bass.AP(
        tensor=src.tensor,
        offset=src.offset,
        ap=[[0, out_sb.partition_size()], [1, n_free]],
    )
    nc.sync.dma_start(out=out_sb, in_=ap)


# ---------------------------------------------------------------------------
# Fast (factored) kernel
# ---------------------------------------------------------------------------

def build_factored(neg_inv_ct, neg_inv_cr, mu_t, mu_r, reps=1, unroll=False,
                   ablate=0):
    # ablate: 0=full, 1=frontend only (stop after gthn), 2=+desc (skip conv)
    # mu_t [NT], mu_r [NR] are baked in as immediates.
    nc = bacc.Bacc(None, target_bir_lowering=False)

    x_d = nc.dram_tensor("x", [SC, NV, 8], F32, kind="ExternalInput")
    # w4[32j+kt, (i, kr, l)] = W_conv[i, kr*16+kt, l], replicated j=0..3 (bf16)
    w_d = nc.dram_tensor("w4", [NV, NF * NR * K], BF16, kind="ExternalInput")
    b_d = nc.dram_tensor("b_c", [NF, K], F32, kind="ExternalInput")
    id_d = nc.dram_tensor("ident", [128, 128], F32, kind="ExternalInput")
    out_d = nc.dram_tensor("out", [SC, K * NF], F32, kind="ExternalOutput")

    with tile.TileContext(nc) as tc:
        with (
            tc.tile_pool(name="const", bufs=1) as const,
            tc.tile_pool(name="work", bufs=2) as work,
            tc.tile_pool(name="gbig", bufs=1) as gbig,
            tc.tile_pool(name="big", bufs=2) as big,
            tc.tile_pool(name="psum", bufs=2, space="PSUM") as psum,
        ):
            # ---- persistent loads (outside the rep loop) ----------------
            x_sb = const.tile([SC, NV * 8], F32)
            nc.sync.dma_start(out=x_sb, in_=x_d[:].rearrange("s v c -> s (v c)"))
            ident = const.tile([128, 128], F32)
            nc.sync.dma_start(out=ident, in_=id_d[:])
            w_sb = const.tile([NV, NF, NR, K], BF16)
            nc.sync.dma_start(
                out=w_sb.rearrange("p i k l -> p (i k l)"), in_=w_d[:]
            )
            b_sb = const.tile([K, NF], F32)
            nc.sync.dma_start(out=b_sb, in_=b_d[:].rearrange("i k -> k i"))
            # per-kt bias columns for the fused (T - mu_t)^2 activation
            mu_tb = const.tile([128, NT], F32)
            for kt in range(NT):
                nc.vector.memset(mu_tb[:, kt : kt + 1], -float(mu_t[kt]))

            def body():
                # ---- transpose per-channel to [NV, SC] -----------------
                xv = x_sb.rearrange("s (v c) -> s v c", c=8)
                featsT = work.tile([NV, NF, SC], F32, tag="featsT")
                rhoT = work.tile([NV, SC], F32, tag="rhoT")
                thetaT = work.tile([NV, SC], F32, tag="thetaT")
                maskT = work.tile([NV, SC], F32, tag="maskT")
                targets = [featsT[:, i, :] for i in range(NF)] + [rhoT, thetaT, maskT]
                for c in range(8):
                    pt = psum.tile([NV, SC], F32, tag="tpsum")
                    nc.tensor.transpose(pt, xv[:, :, c], ident[:SC, :SC])
                    nc.scalar.copy(targets[c], pt)

                # ---- rho gaussians [NV, NR, SC], Sr, recR --------------
                grho = work.tile([NV, NR, SC], F32, tag="grho")
                for kr in range(NR):
                    nc.vector.tensor_scalar_add(grho[:, kr, :], rhoT, -float(mu_r[kr]))
                nc.scalar.activation(grho, grho, AF.Square)
                nc.scalar.activation(grho, grho, AF.Exp, scale=float(neg_inv_cr))
                Sr = work.tile([NV, SC], F32, tag="Sr")
                nc.vector.reduce_sum(Sr, grho.rearrange("v k s -> v s k"), axis=AX.X)
                recR = work.tile([NV, SC], F32, tag="recR")
                nc.vector.reciprocal(recR, Sr)

                # ---- u[v, i, kr, s] = feats*mask*recR*grho  (bf16) -----
                mf = work.tile([NV, NF, SC], F32, tag="mf")
                nc.gpsimd.tensor_mul(mf, featsT, _bcast(maskT, 1, [NV, NF, SC]))
                nc.gpsimd.tensor_mul(mf, mf, _bcast(recR, 1, [NV, NF, SC]))
                u = work.tile([NV, NF, NR, SC], BF16, tag="u")
                nc.gpsimd.tensor_mul(
                    u,
                    _bcast(mf, 2, [NV, NF, NR, SC]),
                    _bcast(grho, 1, [NV, NF, NR, SC]),
                )

                # Pipelined in two rotation halves: the whole theta-gaussian
                # -> normalise -> desc -> conv chain for half 0 overlaps the
                # Act-heavy frontend of half 1.
                HC = R // 2
                gthn = gbig.tile([NV, NT, R, SC], BF16, tag="gthn", bufs=2)
                desc_sb = big.tile([NV, NF, NR, R, SC // 4], BF16, tag="desc",
                                   bufs=2)
                convmax = work.tile([K, NF, SC], F32, tag="convmax")
                cmv = convmax.rearrange("k i (g j) -> k i g j", j=4)

                for h in range(2):
                    r0 = h * HC
                    # T = mod(theta + rot, 2pi) for this half
                    T = work.tile([NV, HC, SC], F32, tag="T")
                    for rr in range(HC):
                        nc.vector.tensor_scalar_add(
                            T[:, rr, :], thetaT, (TWO_PI / R) * (r0 + rr)
                        )
                    ge = work.tile([NV, HC, SC], F32, tag="ge")
                    nc.vector.tensor_scalar(ge, T, TWO_PI, None, ALU.is_ge)
                    nc.vector.scalar_tensor_tensor(
                        T, ge, -TWO_PI, T, ALU.mult, ALU.add
                    )

                    # (T - mu_t)^2 on Act (fused bias); exp -> bf16
                    gsq = gbig.tile([NV, NT, HC, SC], F32, tag="gsq", bufs=2)
                    for kt in range(NT):
                        nc.scalar.activation(
                            gsq[:, kt, :, :], T, AF.Square,
                            bias=mu_tb[:, kt : kt + 1],
                        )
                    gv = gthn[:, :, r0 : r0 + HC, :]
                    nc.scalar.activation(gv, gsq, AF.Exp, scale=float(neg_inv_ct))

                    # Normalise + desc in rotation-quarters so PE desc work
                    # overlaps the DVE tree/recip/mult of later quarters.
                    QC = HC // 2
                    for q in range(2):
                        q0 = r0 + q * QC
                        gq = gthn[:, :, q0 : q0 + QC, :]
                        # St over kt via DVE bf16 add-tree; bf16 fine:
                        # St>=0.96, feeds only a reciprocal (~4e-3 rel).
                        sttmp = gbig.tile([NV, NT // 2, QC, SC], BF16,
                                          tag="sttmp", bufs=2)
                        with nc.allow_low_precision(reason="bf16 St tree"):
                            nc.vector.tensor_add(
                                sttmp, gq[:, : NT // 2], gq[:, NT // 2 :]
                            )
                            n = NT // 2
                            while n > 1:
                                nc.vector.tensor_add(
                                    sttmp[:, : n // 2], sttmp[:, : n // 2],
                                    sttmp[:, n // 2 : n],
                                )
                                n //= 2
                            recT = work.tile([NV, QC, SC], BF16, tag="recT")
                            nc.vector.reciprocal(recT, sttmp[:, 0])
                        # normalise in place: gthn-quarter *= recT (all bf16)
                        nc.vector.tensor_mul(
                            gq, gq, _bcast(recT, 1, [NV, NT, QC, SC])
                        )
                        if ablate == 1:
                            continue

                        # desc: per (r,s) matmul [16kt x 25(i,kr)]; dp tile r
                        # holds all 64 samples of rotation r: band 32j rows
                        # [32j,32j+16) = kt, col group g -> s = 4g+j
                        for rr in range(QC):
                            r = q0 + rr
                            dp = psum.tile([128, (SC // 4) * NF * NR], F32,
                                           tag="dpsum", bufs=3)
                            for g in range(SC // 4):
                                for j in range(4):
                                    s = 4 * g + j
                                    nc.tensor.matmul(
                                        dp[32 * j : 32 * j + NT,
                                           g * NF * NR : (g + 1) * NF * NR],
                                        gthn[:, :, r, s],
                                        u[:, :, :, s],
                                        start=True, stop=True,
                                        tile_position=(0, 32 * j),
                                    )
                            src = dp.rearrange("p (g i k) -> p i k g",
                                               i=NF, k=NR)
                            dst = desc_sb[:, :, :, r, :]
                            if r % 2 == 0:
                                nc.scalar.copy(dst, src)
                            else:
                                nc.vector.tensor_copy(dst, src)

                # ---- conv + max over all rotations ---------------------
                if ablate:
                    nc.vector.memset(convmax, 0.0)
                for i in range(NF) if not ablate else []:
                    for j in range(4):
                        cp = psum.tile([K, R * (SC // 4)], F32, tag="cpsum")
                        for kr in range(NR):
                            nc.tensor.matmul(
                                cp,
                                w_sb[32 * j : 32 * j + NT, i, kr, :],
                                desc_sb[32 * j : 32 * j + NT, i, kr, :, :],
                                start=(kr == 0), stop=(kr == NR - 1),
                                tile_position=(32 * j, 0),
                            )
                        red_in = cp.rearrange("l (r g) -> l g r", r=R)
                        nc.vector.reduce_max(cmv[:, i, :, j], red_in, axis=AX.X)

                # ---- tail: bias + relu, transpose to [SC, K], pack -----
                out_sb = work.tile([SC, K * NF], F32, tag="out_sb", bufs=1)
                oview = out_sb.rearrange("s (k i) -> s k i", i=NF)
                for i in range(NF):
                    act = work.tile([K, SC], F32, tag="act")
                    nc.scalar.activation(
                        act, convmax[:, i, :], AF.Relu, bias=b_sb[:, i : i + 1]
                    )
                    pt = psum.tile([SC, K], F32, tag="opsum", bufs=1)
                    nc.tensor.transpose(pt, act, ident[:K, :K])
                    nc.vector.tensor_copy(oview[:, :, i], pt)
                nc.sync.dma_start(out=out_d[:], in_=out_sb)

            hints = (mybir.EngineType.PE, mybir.EngineType.DVE,
                     mybir.EngineType.Activation, mybir.EngineType.SP,
                     mybir.EngineType.Pool)
            if reps == 1:
                body()
            elif unroll:
                for _ in range(reps):
                    body()
            elif reps % 2 == 0:
                # unroll x2 inside the hardware loop: pool buffer rotation
                # gives adjacent iterations independent tiles, so engines
                # software-pipeline across iterations.
                with tc.For_i(0, reps // 2, 1, hint_engines=hints):
                    body()
                    body()
            else:
                with tc.For_i(0, reps, 1, hint_engines=hints):
                    body()

    return nc


# ---------------------------------------------------------------------------
# Generic fallback kernel (arbitrary mu/sigma): correct, slower
# ---------------------------------------------------------------------------

def build_generic():
    nc = bacc.Bacc(None, target_bir_lowering=False)

    x_d = nc.dram_tensor("x", [SC, NV, 8], F32, kind="ExternalInput")
    # params broadcast-ready, flattened [1, NF*K]
    mu_t_d = nc.dram_tensor("mu_t", [1, NF * K], F32, kind="ExternalInput")
    nict_d = nc.dram_tensor("nict", [1, NF * K], F32, kind="ExternalInput")
    mu_r_d = nc.dram_tensor("mu_r", [1, NF * K], F32, kind="ExternalInput")
    nicr_d = nc.dram_tensor("nicr", [1, NF * K], F32, kind="ExternalInput")
    w_d = nc.dram_tensor("w_c", [NF, K, K], F32, kind="ExternalInput")
    b_d = nc.dram_tensor("b_c", [NF, K], F32, kind="ExternalInput")
    id_d = nc.dram_tensor("ident", [128, 128], F32, kind="ExternalInput")
    out_d = nc.dram_tensor("out", [SC, K * NF], F32, kind="ExternalOutput")

    with tile.TileContext(nc) as tc:
        with (
            tc.tile_pool(name="const", bufs=1) as const,
            tc.tile_pool(name="work", bufs=1) as work,
            tc.tile_pool(name="big", bufs=2) as big,
            tc.tile_pool(name="psum", bufs=2, space="PSUM") as psum,
        ):
            x_sb = const.tile([SC, NV * 8], F32)
            nc.sync.dma_start(out=x_sb, in_=x_d[:].rearrange("s v c -> s (v c)"))
            ident = const.tile([128, 128], F32)
            nc.sync.dma_start(out=ident, in_=id_d[:])
            mu_tb = const.tile([128, NF, K], F32)
            _dma_bcast_rows(nc, mu_tb.rearrange("p i k -> p (i k)"), mu_t_d, NF * K)
            nictb = const.tile([128, NF, K], F32)
            _dma_bcast_rows(nc, nictb.rearrange("p i k -> p (i k)"), nict_d, NF * K)
            mu_rb = const.tile([128, NF, K], F32)
            _dma_bcast_rows(nc, mu_rb.rearrange("p i k -> p (i k)"), mu_r_d, NF * K)
            nicrb = const.tile([128, NF, K], F32)
            _dma_bcast_rows(nc, nicrb.rearrange("p i k -> p (i k)"), nicr_d, NF * K)
            w_sb = const.tile([K, NF, K], F32)
            nc.sync.dma_start(out=w_sb, in_=w_d[:].rearrange("i k l -> k i l"))
            b_sb = const.tile([K, NF], F32)
            nc.sync.dma_start(out=b_sb, in_=b_d[:].rearrange("i k -> k i"))

            xv = x_sb.rearrange("s (v c) -> s v c", c=8)
            featsT = const.tile([NV, NF, SC], F32)
            rhoT = const.tile([NV, SC], F32)
            thetaT = const.tile([NV, SC], F32)
            maskT = const.tile([NV, SC], F32)
            targets = [featsT[:, i, :] for i in range(NF)] + [rhoT, thetaT, maskT]
            for c in range(8):
                pt = psum.tile([NV, SC], F32, tag="tpsum")
                nc.tensor.transpose(pt, xv[:, :, c], ident[:SC, :SC])
                nc.scalar.copy(targets[c], pt)

            T = work.tile([NV, R, SC], F32)
            for r in range(R):
                nc.vector.tensor_scalar_add(T[:, r, :], thetaT, (TWO_PI / R) * r)
            ge = work.tile([NV, R, SC], F32)
            nc.vector.tensor_scalar(ge, T, TWO_PI, None, ALU.is_ge)
            nc.vector.scalar_tensor_tensor(T, ge, -TWO_PI, T, ALU.mult, ALU.add)

            mf = work.tile([NV, NF, SC], F32)
            nc.vector.tensor_mul(mf, featsT, _bcast(maskT, 1, [NV, NF, SC]))

            convmax = const.tile([K, NF, SC], F32)

            for i in range(NF):
                # arg_rho[v, s, k] for this feature
                argr = big.tile([NV, SC, K], F32, tag="argr")
                nc.vector.tensor_sub(
                    argr,
                    _bcast(rhoT, 2, [NV, SC, K]),
                    _bcast(mu_rb[:, i, :], 1, [NV, SC, K]),
                )
                nc.scalar.activation(argr, argr, AF.Square)
                nc.vector.tensor_mul(
                    argr, argr, _bcast(nicrb[:, i, :], 1, [NV, SC, K])
                )
                desc = big.tile([K, R * SC], F32, tag="desc")
                for r in range(R):
                    h = big.tile([NV, SC, K], F32, tag="h")
                    nc.vector.tensor_sub(
                        h,
                        _bcast(T[:, r, :], 2, [NV, SC, K]),
                        _bcast(mu_tb[:, i, :], 1, [NV, SC, K]),
                    )
                    nc.scalar.activation(h, h, AF.Square)
                    nc.vector.tensor_mul(
                        h, h, _bcast(nictb[:, i, :], 1, [NV, SC, K])
                    )
                    nc.vector.tensor_add(h, h, argr)
                    nc.scalar.activation(h, h, AF.Exp)
                    St = work.tile([NV, SC], F32, tag="St")
                    nc.vector.reduce_sum(St, h, axis=AX.X)
                    nc.vector.tensor_scalar_add(St, St, EPS)
                    rec = work.tile([NV, SC], F32, tag="rec")
                    nc.vector.reciprocal(rec, St)
                    wcol = work.tile([NV, SC], F32, tag="wcol")
                    nc.vector.tensor_mul(wcol, mf[:, i, :], rec)
                    dp = psum.tile([K, SC], F32, tag="dpsum")
                    for s in range(SC):
                        nc.tensor.matmul(
                            dp[:, s : s + 1],
                            h[:, s, :],
                            wcol[:, s : s + 1],
                            start=True, stop=True,
                        )
                    nc.scalar.copy(desc[:, r * SC : (r + 1) * SC], dp)

                # conv + max over rotations
                for half in range(2):
                    cp = psum.tile([K, R * SC // 2], F32, tag="cpsum")
                    nc.tensor.matmul(
                        cp,
                        w_sb[:, i, :],
                        desc[:, half * (R * SC // 2) : (half + 1) * (R * SC // 2)],
                        start=True, stop=True,
                    )
                    red = work.tile([K, SC], F32, tag="red")
                    nc.vector.reduce_max(
                        red, cp.rearrange("k (r s) -> k s r", r=R // 2), axis=AX.X
                    )
                    if half == 0:
                        nc.vector.tensor_copy(convmax[:, i, :], red)
                    else:
                        nc.vector.tensor_max(
                            convmax[:, i, :], convmax[:, i, :], red
                        )

            out_sb = const.tile([SC, K * NF], F32)
            oview = out_sb.rearrange("s (k i) -> s k i", i=NF)
            for i in range(NF):
                act = work.tile([K, SC], F32, tag="act")
                nc.scalar.activation(
                    act, convmax[:, i, :], AF.Relu, bias=b_sb[:, i : i + 1]
                )
                pt = psum.tile([SC, K], F32, tag="opsum")
                nc.tensor.transpose(pt, act, ident[:K, :K])
                nc.vector.tensor_copy(oview[:, :, i], pt)
            nc.sync.dma_start(out=out_d[:], in_=out_sb)

    return nc


# ---------------------------------------------------------------------------
# Host driver
# ---------------------------------------------------------------------------

def _detect_factored(mu_rho, sigma_rho, mu_theta, sigma_theta):
    k = np.arange(K)
    kt = k % NT
    kr = (k // NT) * NT
    for a in (mu_rho, sigma_rho, mu_theta, sigma_theta):
        if not np.all(a == a[0:1]):
            return None
    if not (np.array_equal(mu_theta, mu_theta[:, kt])
            and np.array_equal(sigma_theta, sigma_theta[:, kt])
            and np.array_equal(mu_rho, mu_rho[:, kr])
            and np.array_equal(sigma_rho, sigma_rho[:, kr])):
        return None
    c_t = sigma_theta[0, :NT].astype(np.float64) ** 2 + EPS
    c_r = sigma_rho[0, ::NT].astype(np.float64) ** 2 + EPS
    if not (np.all(c_t == c_t[0]) and np.all(c_r == c_r[0])):
        return None
    return {
        "mu_t": mu_theta[0, :NT].copy(),
        "mu_r": mu_rho[0, ::NT].copy(),
        "neg_inv_ct": -1.0 / c_t[0],
        "neg_inv_cr": -1.0 / c_r[0],
    }


_CACHE = {}


def _get_program(key, builder, *args):
    if key not in _CACHE:
        nc = builder(*args)
        if not nc.is_finalized():
            nc.finalize()
        _CACHE[key] = nc
    return _CACHE[key]


def prepare(inputs, reps=1):
    """Build (or fetch cached) program and per-core input maps."""
    x = np.ascontiguousarray(inputs["x"], dtype=np.float32)
    mu_rho = np.asarray(inputs["mu_rho"], dtype=np.float32)
    sigma_rho = np.asarray(inputs["sigma_rho"], dtype=np.float32)
    mu_theta = np.asarray(inputs["mu_theta"], dtype=np.float32)
    sigma_theta = np.asarray(inputs["sigma_theta"], dtype=np.float32)
    W = np.ascontiguousarray(inputs["W_conv"], dtype=np.float32)
    b = np.ascontiguousarray(inputs["b_conv"], dtype=np.float32)

    ident = np.eye(128, dtype=np.float32)
    fact = _detect_factored(mu_rho, sigma_rho, mu_theta, sigma_theta)

    if fact is not None:
        nc = _get_program(
            ("fact2", float(fact["neg_inv_ct"]), float(fact["neg_inv_cr"]),
             fact["mu_t"].tobytes(), fact["mu_r"].tobytes(), reps),
            build_factored, fact["neg_inv_ct"], fact["neg_inv_cr"],
            fact["mu_t"], fact["mu_r"], reps,
        )
        # w4[32j+kt, i, kr, l] = W[i, kr*16+kt, l]; rows 32j+16..32j+32 unused
        wkt = W.reshape(NF, NR, NT, K).transpose(2, 0, 1, 3)  # [kt, i, kr, l]
        w4 = np.zeros((NV, NF, NR, K), dtype=np.float32)
        for j in range(4):
            w4[32 * j : 32 * j + NT] = wkt
        w4 = w4.astype(ml_dtypes.bfloat16).reshape(NV, NF * NR * K)
        common = {
            "w4": w4,
            "b_c": b,
            "ident": ident,
        }
    else:
        nc = _get_program(("gen",), build_generic)
        nict = (-1.0 / (sigma_theta.astype(np.float64) ** 2 + EPS)).astype(np.float32)
        nicr = (-1.0 / (sigma_rho.astype(np.float64) ** 2 + EPS)).astype(np.float32)
        common = {
            "mu_t": mu_theta.reshape(1, NF * K),
            "nict": nict.reshape(1, NF * K),
            "mu_r": mu_rho.reshape(1, NF * K),
            "nicr": nicr.reshape(1, NF * K),
            "w_c": W,
            "b_c": b,
            "ident": ident,
        }

    in_maps = []
    for c in range(N_CORES):
        m = dict(common)
        m["x"] = np.ascontiguousarray(x[c * SC : (c + 1) * SC])
        in_maps.append(m)
    return nc, in_maps


def kernel(**inputs):
    nc, in_maps = prepare(inputs)
    res = run_bass_kernel_spmd(nc, in_maps, core_ids=list(range(N_CORES)))
    return np.concatenate(
        [res.results[c]["out"] for c in range(N_CORES)], axis=0
    )
